# revision 19
# baseline (speedup 1.0000x reference)
"""CoxPH loss kernel for Trainium2, sharded across 8 NeuronCores.

Strategy (per sharding hint): shard the n dimension. Core c owns rows
i in [c*2048, (c+1)*2048). It builds its row-block of the risk-set mask
on the fly in SBUF (never materializing R in HBM):

  per j-chunk of 128 (partitions), one fused DVE tensor_scalar computes
      masked_e[j, i] = (t_loc[i] <= t[j]) * exp_theta[j]
  and the tensor engine reduces over j (partitions) with a ones-vector
  matvec accumulated in PSUM over all 128 j-chunks:
      denom[i] = sum_j masked_e[j, i]

  Then log(denom + 1e-8), the per-row numerator terms, and per-core
  partial sums are computed on-chip. The final all-reduce of the 8
  partial (numerator, event-count) pairs is done on the host.
"""
import sys

sys.path.insert(0, "/opt/trn_rl_repo")

import numpy as np
from contextlib import ExitStack

import concourse.bass as bass
import concourse.tile as tile
from concourse import mybir
from concourse import bass_utils

N = 16384
NCORES = 8
NLOC = N // NCORES  # 2048 rows per core
P = 128
JCH = N // P  # 128 j-chunks
IB = 512  # psum bank width (f32)
NB = NLOC // IB  # 4 psum banks

F32 = mybir.dt.float32
BF16 = mybir.dt.bfloat16
I32 = mybir.dt.int32

USCALE = float(2**24)  # exact power-of-2 time rescale for integer compares
BIGM = 60.0  # Exp(-2*BIGM + theta) flushes to 0; Exp(theta) when sign=+1
ACT_PHASES = (3, 6, 9, 12)  # 4/13 of chunks go to the Scalar engine


def _split_sync_waits(nc, max_waits=1):
    """Walrus's TPB_CTRL NO_STRUCT encoding rejects >1 sync wait per
    instruction (and drains use their wait slots internally); move excess
    waits onto preceding NOPs."""
    for f in nc.m.functions:
        for bb in f.blocks:
            new_insts = []
            for ins in bb.instructions:
                si = getattr(ins, "sync_info", None)
                cap = 0 if type(ins).__name__ == "InstDrain" else max_waits
                if si is not None and si.on_wait and len(si.on_wait) > cap:
                    waits = list(si.on_wait)
                    extra = waits if cap == 0 else waits[:-cap]
                    keep = [] if cap == 0 else waits[-cap:]
                    for i in range(0, len(extra), max_waits):
                        nop = mybir.InstNoOp(
                            name=nc.get_next_instruction_name(),
                            sync_info=mybir.SyncInfo(
                                on_wait=extra[i : i + max_waits], on_update=[]
                            ),
                            bass_nofuse=True,
                            engine=ins.engine,
                        )
                        new_insts.append(nop)
                    si.on_wait = keep
                new_insts.append(ins)
            bb.instructions[:] = new_insts


_prog_cache = {}


def _build_program():
    if "nc" in _prog_cache:
        return _prog_cache["nc"]

    nc = bass.Bass("TRN2", target_bir_lowering=False, debug=False)

    t_full = nc.dram_tensor("t_full", [N], F32, kind="ExternalInput").ap()
    r_full = nc.dram_tensor("r_full", [N], F32, kind="ExternalInput").ap()
    tlb_in = nc.dram_tensor("tlb", [P, NLOC], F32, kind="ExternalInput").ap()
    r_loc = nc.dram_tensor("r_loc", [NLOC], F32, kind="ExternalInput").ap()
    ev_loc = nc.dram_tensor("ev_loc", [NLOC], I32, kind="ExternalInput").ap()
    num_out = nc.dram_tensor("num", [1, 1], F32, kind="ExternalOutput").ap()
    evs_out = nc.dram_tensor("evs", [1, 1], F32, kind="ExternalOutput").ap()

    def pcol(ap_1d, n_free):
        # [n] dram -> [128, n_free] row-major: out[p, c] = in[p*n_free + c].
        # (contiguous per partition — chunk ci is the strided j-set
        # {p*n_free + ci}, which is fine: any disjoint cover of j works)
        return bass.AP(
            tensor=ap_1d.tensor, offset=ap_1d.offset, ap=[[n_free, P], [1, n_free]]
        )

    def bcast(ap_1d, n_free):
        # [n] dram -> [128, n_free] broadcast along partitions
        return bass.AP(
            tensor=ap_1d.tensor, offset=ap_1d.offset, ap=[[0, P], [1, n_free]]
        )

    def row(ap_1d, n_free):
        # [n] dram -> [1, n_free] in partition 0
        return bass.AP(
            tensor=ap_1d.tensor, offset=ap_1d.offset, ap=[[0, 1], [1, n_free]]
        )

    with tile.TileContext(nc) as tc, ExitStack() as ctx:
        singles = ctx.enter_context(tc.tile_pool(name="singles", bufs=1))
        mask_pool = ctx.enter_context(tc.tile_pool(name="mask", bufs=6))
        sign_pool = ctx.enter_context(tc.tile_pool(name="sign", bufs=2))
        psum_pool = ctx.enter_context(tc.tile_pool(name="psum", bufs=1, space="PSUM"))

        # --- stage 0: load + prep -------------------------------------
        t_sb = singles.tile([P, JCH], F32)
        nc.sync.dma_start(t_sb[:], pcol(t_full, JCH))

        r_sb = singles.tile([P, JCH], F32)
        nc.sync.dma_start(r_sb[:], pcol(r_full, JCH))

        th_sb = singles.tile([P, JCH], F32)
        # theta = clip(risk, -20, 20); e = exp(theta)
        nc.vector.tensor_scalar(
            th_sb[:], r_sb[:], -20.0, 20.0, mybir.AluOpType.max, mybir.AluOpType.min
        )
        e_sb = singles.tile([P, JCH], F32)
        nc.scalar.activation(e_sb[:], th_sb[:], mybir.ActivationFunctionType.Exp)
        # thm = theta - BIGM, for the ACT-path Exp(a*BIGM + theta - BIGM)
        thm_sb = singles.tile([P, JCH], F32)
        nc.vector.tensor_scalar_add(thm_sb[:], th_sb[:], -BIGM)

        # u = t * 2^24: exact power-of-2 rescale; jax uniform t lies on a
        # 2^-24-grid so u-values and their differences are exact integers
        # in f32. The ACT path uses a = Sign(u_j + 1 - u_i), mapped through
        # Exp(a*BIGM + theta_j - BIGM): a=+1 (u_j >= u_i, ties included)
        # -> e_j; a=0 (u_j == u_i - 1) -> e^(theta-60) ~ 1e-26 ~ 0; a=-1
        # -> underflow 0. u_j + 1 <= 2^24 stays exactly representable.
        ub_sb = singles.tile([P, JCH], F32)
        nc.vector.tensor_scalar(
            ub_sb[:], t_sb[:], USCALE, 1.0, mybir.AluOpType.mult, mybir.AluOpType.add
        )
        u_sb = singles.tile([P, JCH], F32)
        nc.vector.tensor_scalar_mul(u_sb[:], t_sb[:], USCALE)

        # ulb[p, i] = u_loc[i] (host passes time_slice * 2^24, replicated)
        ulb = singles.tile([P, NLOC], F32)
        nc.sync.dma_start(ulb[:], tlb_in[:])

        ones_sb = singles.tile([P, 1], BF16)
        nc.vector.memset(ones_sb[:], 1.0)

        rloc_row = singles.tile([1, NLOC], F32)
        nc.sync.dma_start(rloc_row[:], row(r_loc, NLOC))
        theta_row = singles.tile([1, NLOC], F32)
        nc.vector.tensor_scalar(
            theta_row[:],
            rloc_row[:],
            -20.0,
            20.0,
            mybir.AluOpType.max,
            mybir.AluOpType.min,
        )

        evi_row = singles.tile([1, NLOC], I32)
        nc.sync.dma_start(evi_row[:], row(ev_loc, NLOC))
        evf_row = singles.tile([1, NLOC], F32)
        nc.vector.tensor_copy(evf_row[:], evi_row[:])

        # s1 = sum(theta * ev), evs = sum(ev), via fused accumulate ops
        thev_row = singles.tile([1, NLOC], F32)
        s1_sb = singles.tile([1, 1], F32)
        nc.vector.scalar_tensor_tensor(
            thev_row[:],
            theta_row[:],
            1.0,
            evf_row[:],
            mybir.AluOpType.mult,
            mybir.AluOpType.mult,
            accum_out=s1_sb[:],
        )
        evcopy_row = singles.tile([1, NLOC], F32)
        evs_sb = singles.tile([1, 1], F32)
        nc.vector.tensor_scalar(
            evcopy_row[:],
            evf_row[:],
            0.0,
            None,
            mybir.AluOpType.add,
            mybir.AluOpType.add,
            accum_out=evs_sb[:],
        )

        # --- stage 1: masked accumulation over all j ------------------
        # DVE chunks: one fused tensor_scalar; ACT chunks: Sign + Exp pair
        psums = [psum_pool.tile([1, IB], F32, name=f"psum{i}") for i in range(NB)]
        for ci in range(JCH):
            me = mask_pool.tile([P, NLOC], BF16)
            if (ci % 13) in ACT_PHASES:
                sg = sign_pool.tile([P, NLOC], BF16)
                nc.scalar.activation(
                    sg[:],
                    ulb[:],
                    mybir.ActivationFunctionType.Sign,
                    bias=ub_sb[:, ci : ci + 1],
                    scale=-1.0,
                )
                nc.scalar.activation(
                    me[:],
                    sg[:],
                    mybir.ActivationFunctionType.Exp,
                    bias=thm_sb[:, ci : ci + 1],
                    scale=BIGM,
                )
            else:
                nc.vector.tensor_scalar(
                    me[:],
                    ulb[:],
                    u_sb[:, ci : ci + 1],
                    e_sb[:, ci : ci + 1],
                    mybir.AluOpType.is_le,
                    mybir.AluOpType.mult,
                )
            for ib in range(NB):
                nc.tensor.matmul(
                    psums[ib][:],
                    lhsT=ones_sb[:],
                    rhs=me[:, ib * IB : (ib + 1) * IB],
                    start=(ci == 0),
                    stop=(ci == JCH - 1),
                )

        # --- stage 2: log-denominator + partial loss ------------------
        # sum_i ev_i*log(denom_i + eps) == sum_i log((denom_i - 1)*ev_i + 1)
        # (+eps shift is below f32 resolution here; denom >= e^-20 + ~1)
        one_sb = singles.tile([1, 1], F32)
        nc.vector.memset(one_sb[:], 1.0)
        y_row = singles.tile([1, NLOC], F32)
        ls_sb = singles.tile([1, NB], F32)
        for ib in range(NB):
            nc.vector.scalar_tensor_tensor(
                y_row[:, ib * IB : (ib + 1) * IB],
                psums[ib][:],
                1.0,
                evf_row[:, ib * IB : (ib + 1) * IB],
                mybir.AluOpType.subtract,
                mybir.AluOpType.mult,
            )
            nc.scalar.activation(
                y_row[:, ib * IB : (ib + 1) * IB],
                y_row[:, ib * IB : (ib + 1) * IB],
                mybir.ActivationFunctionType.Ln,
                bias=one_sb[:],
                accum_out=ls_sb[:, ib : ib + 1],
            )

        sumlog_sb = singles.tile([1, 1], F32)
        nc.vector.tensor_reduce(
            sumlog_sb[:], ls_sb[:], mybir.AxisListType.X, mybir.AluOpType.add
        )
        num_sb = singles.tile([1, 1], F32)
        nc.vector.tensor_tensor(
            num_sb[:], s1_sb[:], sumlog_sb[:], mybir.AluOpType.subtract
        )

        nc.sync.dma_start(num_out[:], num_sb[:])
        nc.sync.dma_start(evs_out[:], evs_sb[:])

    _prog_cache["nc"] = nc
    return nc


def kernel(risk, time, event):
    risk = np.asarray(risk, dtype=np.float32)
    time = np.asarray(time, dtype=np.float32)
    event = np.asarray(event, dtype=np.int32)

    nc = _build_program()
    if "fixed" not in _prog_cache:
        _split_sync_waits(nc)
        _prog_cache["fixed"] = True

    in_maps = []
    for c in range(NCORES):
        s = slice(c * NLOC, (c + 1) * NLOC)
        in_maps.append(
            {
                "t_full": time,
                "r_full": risk,
                "tlb": np.ascontiguousarray(
                    np.broadcast_to(time[s] * np.float32(USCALE), (P, NLOC))
                ),
                "r_loc": risk[s].copy(),
                "ev_loc": event[s].copy(),
            }
        )

    res = bass_utils.run_bass_kernel_spmd(nc, in_maps, core_ids=list(range(NCORES)))

    num = sum(float(res.results[c]["num"][0, 0]) for c in range(NCORES))
    evs = sum(float(res.results[c]["evs"][0, 0]) for c in range(NCORES))
    return np.float32(-(num / (evs + 1e-8)))


def _make_in_maps(risk, time, event):
    in_maps = []
    for c in range(NCORES):
        s = slice(c * NLOC, (c + 1) * NLOC)
        in_maps.append(
            {
                "t_full": time,
                "r_full": risk,
                "tlb": np.ascontiguousarray(
                    np.broadcast_to(time[s] * np.float32(USCALE), (P, NLOC))
                ),
                "r_loc": risk[s].copy(),
                "ev_loc": event[s].copy(),
            }
        )
    return in_maps


def profile(np_inputs, tmpdir=None):
    """Run once with NTFF tracing; returns exec_time_ns (max across cores)."""
    risk = np.asarray(np_inputs["risk"], dtype=np.float32)
    time = np.asarray(np_inputs["time"], dtype=np.float32)
    event = np.asarray(np_inputs["event"], dtype=np.int32)
    nc = _build_program()
    if "fixed" not in _prog_cache:
        _split_sync_waits(nc)
        _prog_cache["fixed"] = True
    res = bass_utils.run_bass_kernel_spmd(
        nc,
        _make_in_maps(risk, time, event),
        core_ids=list(range(NCORES)),
        trace=True,
        tmpdir=tmpdir,
    )
    if res.instructions_and_trace is not None:
        print("trace:", res.instructions_and_trace[1])
    print("mean_exec_time_ns:", res.mean_exec_time_ns,
          "max core:", res.max_exec_time_core_id)
    return res.exec_time_ns


# revision 20
# speedup vs baseline: 1.1481x; 1.1481x over previous
"""CoxPH loss kernel for Trainium2, sharded across 8 NeuronCores.

Strategy (per sharding hint): shard the n dimension. Core c owns rows
i in [c*2048, (c+1)*2048). It builds its row-block of the risk-set mask
on the fly in SBUF (never materializing R in HBM):

  per j-chunk of 128 (partitions), one fused DVE tensor_scalar computes
      masked_e[j, i] = (t_loc[i] <= t[j]) * exp_theta[j]
  and the tensor engine reduces over j (partitions) with a ones-vector
  matvec accumulated in PSUM over all 128 j-chunks:
      denom[i] = sum_j masked_e[j, i]

  Then log(denom + 1e-8), the per-row numerator terms, and per-core
  partial sums are computed on-chip. The final all-reduce of the 8
  partial (numerator, event-count) pairs is done on the host.
"""
import sys

sys.path.insert(0, "/opt/trn_rl_repo")

import numpy as np
from contextlib import ExitStack

import concourse.bass as bass
import concourse.tile as tile
from concourse import mybir
from concourse import bass_utils

N = 16384
NCORES = 8
NLOC = N // NCORES  # 2048 rows per core
P = 128
JCH = N // P  # 128 j-chunks
IB = 512  # psum bank width (f32)
NB = NLOC // IB  # 4 psum banks

F32 = mybir.dt.float32
BF16 = mybir.dt.bfloat16
I32 = mybir.dt.int32

USCALE = float(2**24)  # exact power-of-2 time rescale for integer compares
BIGM = 60.0  # Exp(-2*BIGM + theta) flushes to 0; Exp(theta) when sign=+1
ACT_PHASES = (1,)  # of every 4 chunks, 1 goes to the Scalar engine


def _split_sync_waits(nc, max_waits=1):
    """Walrus's TPB_CTRL NO_STRUCT encoding rejects >1 sync wait per
    instruction (and drains use their wait slots internally); move excess
    waits onto preceding NOPs."""
    for f in nc.m.functions:
        for bb in f.blocks:
            new_insts = []
            for ins in bb.instructions:
                si = getattr(ins, "sync_info", None)
                cap = 0 if type(ins).__name__ == "InstDrain" else max_waits
                if si is not None and si.on_wait and len(si.on_wait) > cap:
                    waits = list(si.on_wait)
                    extra = waits if cap == 0 else waits[:-cap]
                    keep = [] if cap == 0 else waits[-cap:]
                    for i in range(0, len(extra), max_waits):
                        nop = mybir.InstNoOp(
                            name=nc.get_next_instruction_name(),
                            sync_info=mybir.SyncInfo(
                                on_wait=extra[i : i + max_waits], on_update=[]
                            ),
                            bass_nofuse=True,
                            engine=ins.engine,
                        )
                        new_insts.append(nop)
                    si.on_wait = keep
                new_insts.append(ins)
            bb.instructions[:] = new_insts


_prog_cache = {}


def _build_program():
    if "nc" in _prog_cache:
        return _prog_cache["nc"]

    nc = bass.Bass("TRN2", target_bir_lowering=False, debug=False)

    t_full = nc.dram_tensor("t_full", [N], F32, kind="ExternalInput").ap()
    r_full = nc.dram_tensor("r_full", [N], F32, kind="ExternalInput").ap()
    tlb_in = nc.dram_tensor("tlb", [P, NLOC], F32, kind="ExternalInput").ap()
    r_loc = nc.dram_tensor("r_loc", [NLOC], F32, kind="ExternalInput").ap()
    ev_loc = nc.dram_tensor("ev_loc", [NLOC], I32, kind="ExternalInput").ap()
    num_out = nc.dram_tensor("num", [1, 1], F32, kind="ExternalOutput").ap()
    evs_out = nc.dram_tensor("evs", [1, 1], F32, kind="ExternalOutput").ap()

    def pcol(ap_1d, n_free):
        # [n] dram -> [128, n_free] row-major: out[p, c] = in[p*n_free + c].
        # (contiguous per partition — chunk ci is the strided j-set
        # {p*n_free + ci}, which is fine: any disjoint cover of j works)
        return bass.AP(
            tensor=ap_1d.tensor, offset=ap_1d.offset, ap=[[n_free, P], [1, n_free]]
        )

    def bcast(ap_1d, n_free):
        # [n] dram -> [128, n_free] broadcast along partitions
        return bass.AP(
            tensor=ap_1d.tensor, offset=ap_1d.offset, ap=[[0, P], [1, n_free]]
        )

    def row(ap_1d, n_free):
        # [n] dram -> [1, n_free] in partition 0
        return bass.AP(
            tensor=ap_1d.tensor, offset=ap_1d.offset, ap=[[0, 1], [1, n_free]]
        )

    with tile.TileContext(nc) as tc, ExitStack() as ctx:
        singles = ctx.enter_context(tc.tile_pool(name="singles", bufs=1))
        mask_pool = ctx.enter_context(tc.tile_pool(name="mask", bufs=6))
        sign_pool = ctx.enter_context(tc.tile_pool(name="sign", bufs=2))
        psum_pool = ctx.enter_context(tc.tile_pool(name="psum", bufs=1, space="PSUM"))

        # --- stage 0: load + prep -------------------------------------
        t_sb = singles.tile([P, JCH], F32)
        nc.sync.dma_start(t_sb[:], pcol(t_full, JCH))

        r_sb = singles.tile([P, JCH], F32)
        nc.sync.dma_start(r_sb[:], pcol(r_full, JCH))

        th_sb = singles.tile([P, JCH], F32)
        # theta = clip(risk, -20, 20); e = exp(theta)
        nc.vector.tensor_scalar(
            th_sb[:], r_sb[:], -20.0, 20.0, mybir.AluOpType.max, mybir.AluOpType.min
        )
        e_sb = singles.tile([P, JCH], F32)
        nc.scalar.activation(e_sb[:], th_sb[:], mybir.ActivationFunctionType.Exp)
        # thm = theta - BIGM, for the ACT-path Exp(a*BIGM + theta - BIGM)
        thm_sb = singles.tile([P, JCH], F32)
        nc.vector.tensor_scalar_add(thm_sb[:], th_sb[:], -BIGM)

        # u = t * 2^24: exact power-of-2 rescale; jax uniform t lies on a
        # 2^-24-grid so u-values and their differences are exact integers
        # in f32. The ACT path uses a = Sign(u_j + 1 - u_i), mapped through
        # Exp(a*BIGM + theta_j - BIGM): a=+1 (u_j >= u_i, ties included)
        # -> e_j; a=0 (u_j == u_i - 1) -> e^(theta-60) ~ 1e-26 ~ 0; a=-1
        # -> underflow 0. u_j + 1 <= 2^24 stays exactly representable.
        ub_sb = singles.tile([P, JCH], F32)
        nc.vector.tensor_scalar(
            ub_sb[:], t_sb[:], USCALE, 1.0, mybir.AluOpType.mult, mybir.AluOpType.add
        )
        u_sb = singles.tile([P, JCH], F32)
        nc.vector.tensor_scalar_mul(u_sb[:], t_sb[:], USCALE)

        # ulb[p, i] = u_loc[i] (host passes time_slice * 2^24, replicated)
        ulb = singles.tile([P, NLOC], F32)
        nc.sync.dma_start(ulb[:], tlb_in[:])

        ones_sb = singles.tile([P, 1], BF16)
        nc.vector.memset(ones_sb[:], 1.0)

        # --- stage 1: masked accumulation over all j ------------------
        # DVE chunks: one fused tensor_scalar; ACT chunks: Sign + Exp pair
        psums = [psum_pool.tile([1, IB], F32, name=f"psum{i}") for i in range(NB)]
        for ci in range(JCH):
            me = mask_pool.tile([P, NLOC], BF16)
            if (ci % 4) in ACT_PHASES:
                sg = sign_pool.tile([P, NLOC], BF16)
                nc.scalar.activation(
                    sg[:],
                    ulb[:],
                    mybir.ActivationFunctionType.Sign,
                    bias=ub_sb[:, ci : ci + 1],
                    scale=-1.0,
                )
                nc.scalar.activation(
                    me[:],
                    sg[:],
                    mybir.ActivationFunctionType.Exp,
                    bias=thm_sb[:, ci : ci + 1],
                    scale=BIGM,
                )
            else:
                nc.vector.tensor_scalar(
                    me[:],
                    ulb[:],
                    u_sb[:, ci : ci + 1],
                    e_sb[:, ci : ci + 1],
                    mybir.AluOpType.is_le,
                    mybir.AluOpType.mult,
                )
            for ib in range(NB):
                nc.tensor.matmul(
                    psums[ib][:],
                    lhsT=ones_sb[:],
                    rhs=me[:, ib * IB : (ib + 1) * IB],
                    start=(ci == 0),
                    stop=(ci == JCH - 1),
                )

        rloc_row = singles.tile([1, NLOC], F32)
        nc.sync.dma_start(rloc_row[:], row(r_loc, NLOC))
        theta_row = singles.tile([1, NLOC], F32)
        nc.vector.tensor_scalar(
            theta_row[:],
            rloc_row[:],
            -20.0,
            20.0,
            mybir.AluOpType.max,
            mybir.AluOpType.min,
        )

        evi_row = singles.tile([1, NLOC], I32)
        nc.sync.dma_start(evi_row[:], row(ev_loc, NLOC))
        evf_row = singles.tile([1, NLOC], F32)
        nc.vector.tensor_copy(evf_row[:], evi_row[:])

        # s1 = sum(theta * ev), evs = sum(ev), via fused accumulate ops
        thev_row = singles.tile([1, NLOC], F32)
        s1_sb = singles.tile([1, 1], F32)
        nc.vector.scalar_tensor_tensor(
            thev_row[:],
            theta_row[:],
            1.0,
            evf_row[:],
            mybir.AluOpType.mult,
            mybir.AluOpType.mult,
            accum_out=s1_sb[:],
        )
        evcopy_row = singles.tile([1, NLOC], F32)
        evs_sb = singles.tile([1, 1], F32)
        nc.vector.tensor_scalar(
            evcopy_row[:],
            evf_row[:],
            0.0,
            None,
            mybir.AluOpType.add,
            mybir.AluOpType.add,
            accum_out=evs_sb[:],
        )

        # --- stage 2: log-denominator + partial loss ------------------
        # sum_i ev_i*log(denom_i + eps) == sum_i log((denom_i - 1)*ev_i + 1)
        # (+eps shift is below f32 resolution here; denom >= e^-20 + ~1)
        one_sb = singles.tile([1, 1], F32)
        nc.vector.memset(one_sb[:], 1.0)
        y_row = singles.tile([1, NLOC], F32)
        ls_sb = singles.tile([1, NB], F32)
        for ib in range(NB):
            nc.vector.scalar_tensor_tensor(
                y_row[:, ib * IB : (ib + 1) * IB],
                psums[ib][:],
                1.0,
                evf_row[:, ib * IB : (ib + 1) * IB],
                mybir.AluOpType.subtract,
                mybir.AluOpType.mult,
            )
            nc.scalar.activation(
                y_row[:, ib * IB : (ib + 1) * IB],
                y_row[:, ib * IB : (ib + 1) * IB],
                mybir.ActivationFunctionType.Ln,
                bias=one_sb[:],
                accum_out=ls_sb[:, ib : ib + 1],
            )

        sumlog_sb = singles.tile([1, 1], F32)
        nc.vector.tensor_reduce(
            sumlog_sb[:], ls_sb[:], mybir.AxisListType.X, mybir.AluOpType.add
        )
        num_sb = singles.tile([1, 1], F32)
        nc.vector.tensor_tensor(
            num_sb[:], s1_sb[:], sumlog_sb[:], mybir.AluOpType.subtract
        )

        nc.sync.dma_start(num_out[:], num_sb[:])
        nc.sync.dma_start(evs_out[:], evs_sb[:])

    _prog_cache["nc"] = nc
    return nc


def kernel(risk, time, event):
    risk = np.asarray(risk, dtype=np.float32)
    time = np.asarray(time, dtype=np.float32)
    event = np.asarray(event, dtype=np.int32)

    nc = _build_program()
    if "fixed" not in _prog_cache:
        _split_sync_waits(nc)
        _prog_cache["fixed"] = True

    in_maps = []
    for c in range(NCORES):
        s = slice(c * NLOC, (c + 1) * NLOC)
        in_maps.append(
            {
                "t_full": time,
                "r_full": risk,
                "tlb": np.ascontiguousarray(
                    np.broadcast_to(time[s] * np.float32(USCALE), (P, NLOC))
                ),
                "r_loc": risk[s].copy(),
                "ev_loc": event[s].copy(),
            }
        )

    res = bass_utils.run_bass_kernel_spmd(nc, in_maps, core_ids=list(range(NCORES)))

    num = sum(float(res.results[c]["num"][0, 0]) for c in range(NCORES))
    evs = sum(float(res.results[c]["evs"][0, 0]) for c in range(NCORES))
    return np.float32(-(num / (evs + 1e-8)))


def _make_in_maps(risk, time, event):
    in_maps = []
    for c in range(NCORES):
        s = slice(c * NLOC, (c + 1) * NLOC)
        in_maps.append(
            {
                "t_full": time,
                "r_full": risk,
                "tlb": np.ascontiguousarray(
                    np.broadcast_to(time[s] * np.float32(USCALE), (P, NLOC))
                ),
                "r_loc": risk[s].copy(),
                "ev_loc": event[s].copy(),
            }
        )
    return in_maps


def profile(np_inputs, tmpdir=None):
    """Run once with NTFF tracing; returns exec_time_ns (max across cores)."""
    risk = np.asarray(np_inputs["risk"], dtype=np.float32)
    time = np.asarray(np_inputs["time"], dtype=np.float32)
    event = np.asarray(np_inputs["event"], dtype=np.int32)
    nc = _build_program()
    if "fixed" not in _prog_cache:
        _split_sync_waits(nc)
        _prog_cache["fixed"] = True
    res = bass_utils.run_bass_kernel_spmd(
        nc,
        _make_in_maps(risk, time, event),
        core_ids=list(range(NCORES)),
        trace=True,
        tmpdir=tmpdir,
    )
    if res.instructions_and_trace is not None:
        print("trace:", res.instructions_and_trace[1])
    print("mean_exec_time_ns:", res.mean_exec_time_ns,
          "max core:", res.max_exec_time_core_id)
    return res.exec_time_ns


# revision 22
# speedup vs baseline: 1.1595x; 1.0100x over previous
"""CoxPH loss kernel for Trainium2, sharded across 8 NeuronCores.

Strategy (per sharding hint): shard the n dimension. Core c owns rows
i in [c*2048, (c+1)*2048). It builds its row-block of the risk-set mask
on the fly in SBUF (never materializing R in HBM):

  per j-chunk of 128 (partitions), one fused DVE tensor_scalar computes
      masked_e[j, i] = (t_loc[i] <= t[j]) * exp_theta[j]
  and the tensor engine reduces over j (partitions) with a ones-vector
  matvec accumulated in PSUM over all 128 j-chunks:
      denom[i] = sum_j masked_e[j, i]

  Then log(denom + 1e-8), the per-row numerator terms, and per-core
  partial sums are computed on-chip. The final all-reduce of the 8
  partial (numerator, event-count) pairs is done on the host.
"""
import sys

sys.path.insert(0, "/opt/trn_rl_repo")

import numpy as np
from contextlib import ExitStack

import concourse.bass as bass
import concourse.tile as tile
from concourse import mybir
from concourse import bass_utils

N = 16384
NCORES = 8
NLOC = N // NCORES  # 2048 rows per core
P = 128
JCH = N // P  # 128 j-chunks
IB = 512  # psum bank width (f32)
NB = NLOC // IB  # 4 psum banks

F32 = mybir.dt.float32
BF16 = mybir.dt.bfloat16
I32 = mybir.dt.int32

USCALE = float(2**24)  # exact power-of-2 time rescale for integer compares
BIGM = 60.0  # Exp(-2*BIGM + theta) flushes to 0; Exp(theta) when sign=+1
ACT_PHASES = (1,)  # of every 4 chunks, 1 goes to the Scalar engine


def _split_sync_waits(nc, max_waits=1):
    """Walrus's TPB_CTRL NO_STRUCT encoding rejects >1 sync wait per
    instruction (and drains use their wait slots internally); move excess
    waits onto preceding NOPs."""
    for f in nc.m.functions:
        for bb in f.blocks:
            new_insts = []
            for ins in bb.instructions:
                si = getattr(ins, "sync_info", None)
                cap = 0 if type(ins).__name__ == "InstDrain" else max_waits
                if si is not None and si.on_wait and len(si.on_wait) > cap:
                    waits = list(si.on_wait)
                    extra = waits if cap == 0 else waits[:-cap]
                    keep = [] if cap == 0 else waits[-cap:]
                    for i in range(0, len(extra), max_waits):
                        nop = mybir.InstNoOp(
                            name=nc.get_next_instruction_name(),
                            sync_info=mybir.SyncInfo(
                                on_wait=extra[i : i + max_waits], on_update=[]
                            ),
                            bass_nofuse=True,
                            engine=ins.engine,
                        )
                        new_insts.append(nop)
                    si.on_wait = keep
                new_insts.append(ins)
            bb.instructions[:] = new_insts


def _hoist_input_dmas(nc):
    """Move wait-free input DMA triggers ahead of the preamble barrier so
    the HW DMA queues fill SBUF while the engines synchronize (~3us)."""
    f = nc.m.functions[0]
    main_bb, body_bb = f.blocks[0], f.blocks[1]
    moved = []
    kept = []
    for pos, ins in enumerate(body_bb.instructions):
        si = getattr(ins, "sync_info", None)
        if (
            pos < 25
            and len(moved) < 8
            and type(ins).__name__ == "InstDMACopy"
            and ins.engine == mybir.EngineType.SP
            and not (si and si.on_wait)
        ):
            moved.append(ins)
        else:
            kept.append(ins)
    body_bb.instructions[:] = kept
    idx = next(
        i
        for i, ins in enumerate(main_bb.instructions)
        if type(ins).__name__ == "InstDrain" and ins.engine == mybir.EngineType.SP
    )
    main_bb.instructions[idx:idx] = moved


_prog_cache = {}


def _finalize(nc):
    if "fixed" not in _prog_cache:
        _hoist_input_dmas(nc)
        _split_sync_waits(nc)
        _prog_cache["fixed"] = True


def _build_program():
    if "nc" in _prog_cache:
        return _prog_cache["nc"]

    nc = bass.Bass("TRN2", target_bir_lowering=False, debug=False)

    t_full = nc.dram_tensor("t_full", [N], F32, kind="ExternalInput").ap()
    r_full = nc.dram_tensor("r_full", [N], F32, kind="ExternalInput").ap()
    tlb_in = nc.dram_tensor("tlb", [P, NLOC], F32, kind="ExternalInput").ap()
    r_loc = nc.dram_tensor("r_loc", [NLOC], F32, kind="ExternalInput").ap()
    ev_loc = nc.dram_tensor("ev_loc", [NLOC], I32, kind="ExternalInput").ap()
    num_out = nc.dram_tensor("num", [1, 1], F32, kind="ExternalOutput").ap()
    evs_out = nc.dram_tensor("evs", [1, 1], F32, kind="ExternalOutput").ap()

    def pcol(ap_1d, n_free):
        # [n] dram -> [128, n_free] row-major: out[p, c] = in[p*n_free + c].
        # (contiguous per partition — chunk ci is the strided j-set
        # {p*n_free + ci}, which is fine: any disjoint cover of j works)
        return bass.AP(
            tensor=ap_1d.tensor, offset=ap_1d.offset, ap=[[n_free, P], [1, n_free]]
        )

    def bcast(ap_1d, n_free):
        # [n] dram -> [128, n_free] broadcast along partitions
        return bass.AP(
            tensor=ap_1d.tensor, offset=ap_1d.offset, ap=[[0, P], [1, n_free]]
        )

    def row(ap_1d, n_free):
        # [n] dram -> [1, n_free] in partition 0
        return bass.AP(
            tensor=ap_1d.tensor, offset=ap_1d.offset, ap=[[0, 1], [1, n_free]]
        )

    with tile.TileContext(nc) as tc, ExitStack() as ctx:
        singles = ctx.enter_context(tc.tile_pool(name="singles", bufs=1))
        mask_pool = ctx.enter_context(tc.tile_pool(name="mask", bufs=6))
        sign_pool = ctx.enter_context(tc.tile_pool(name="sign", bufs=2))
        psum_pool = ctx.enter_context(tc.tile_pool(name="psum", bufs=1, space="PSUM"))

        # --- stage 0: load + prep -------------------------------------
        t_sb = singles.tile([P, JCH], F32)
        nc.sync.dma_start(t_sb[:], pcol(t_full, JCH))

        r_sb = singles.tile([P, JCH], F32)
        nc.sync.dma_start(r_sb[:], pcol(r_full, JCH))

        th_sb = singles.tile([P, JCH], F32)
        # theta = clip(risk, -20, 20); e = exp(theta)
        nc.vector.tensor_scalar(
            th_sb[:], r_sb[:], -20.0, 20.0, mybir.AluOpType.max, mybir.AluOpType.min
        )
        e_sb = singles.tile([P, JCH], F32)
        nc.scalar.activation(e_sb[:], th_sb[:], mybir.ActivationFunctionType.Exp)
        # thm = theta - BIGM, for the ACT-path Exp(a*BIGM + theta - BIGM)
        thm_sb = singles.tile([P, JCH], F32)
        nc.vector.tensor_scalar_add(thm_sb[:], th_sb[:], -BIGM)

        # u = t * 2^24: exact power-of-2 rescale; jax uniform t lies on a
        # 2^-24-grid so u-values and their differences are exact integers
        # in f32. The ACT path uses a = Sign(u_j + 1 - u_i), mapped through
        # Exp(a*BIGM + theta_j - BIGM): a=+1 (u_j >= u_i, ties included)
        # -> e_j; a=0 (u_j == u_i - 1) -> e^(theta-60) ~ 1e-26 ~ 0; a=-1
        # -> underflow 0. u_j + 1 <= 2^24 stays exactly representable.
        ub_sb = singles.tile([P, JCH], F32)
        nc.vector.tensor_scalar(
            ub_sb[:], t_sb[:], USCALE, 1.0, mybir.AluOpType.mult, mybir.AluOpType.add
        )
        u_sb = singles.tile([P, JCH], F32)
        nc.vector.tensor_scalar_mul(u_sb[:], t_sb[:], USCALE)

        # ulb[p, i] = u_loc[i] (host passes time_slice * 2^24, replicated)
        ulb = singles.tile([P, NLOC], F32)
        nc.sync.dma_start(ulb[:], tlb_in[:])

        ones_sb = singles.tile([P, 1], BF16)
        nc.vector.memset(ones_sb[:], 1.0)

        # --- stage 1: masked accumulation over all j ------------------
        # DVE chunks: one fused tensor_scalar; ACT chunks: Sign + Exp pair
        psums = [psum_pool.tile([1, IB], F32, name=f"psum{i}") for i in range(NB)]
        for ci in range(JCH):
            me = mask_pool.tile([P, NLOC], BF16)
            if (ci % 4) in ACT_PHASES:
                sg = sign_pool.tile([P, NLOC], BF16)
                nc.scalar.activation(
                    sg[:],
                    ulb[:],
                    mybir.ActivationFunctionType.Sign,
                    bias=ub_sb[:, ci : ci + 1],
                    scale=-1.0,
                )
                nc.scalar.activation(
                    me[:],
                    sg[:],
                    mybir.ActivationFunctionType.Exp,
                    bias=thm_sb[:, ci : ci + 1],
                    scale=BIGM,
                )
            else:
                nc.vector.tensor_scalar(
                    me[:],
                    ulb[:],
                    u_sb[:, ci : ci + 1],
                    e_sb[:, ci : ci + 1],
                    mybir.AluOpType.is_le,
                    mybir.AluOpType.mult,
                )
            for ib in range(NB):
                nc.tensor.matmul(
                    psums[ib][:],
                    lhsT=ones_sb[:],
                    rhs=me[:, ib * IB : (ib + 1) * IB],
                    start=(ci == 0),
                    stop=(ci == JCH - 1),
                )

        rloc_row = singles.tile([1, NLOC], F32)
        nc.sync.dma_start(rloc_row[:], row(r_loc, NLOC))
        theta_row = singles.tile([1, NLOC], F32)
        nc.vector.tensor_scalar(
            theta_row[:],
            rloc_row[:],
            -20.0,
            20.0,
            mybir.AluOpType.max,
            mybir.AluOpType.min,
        )

        evi_row = singles.tile([1, NLOC], I32)
        nc.sync.dma_start(evi_row[:], row(ev_loc, NLOC))
        evf_row = singles.tile([1, NLOC], F32)
        nc.vector.tensor_copy(evf_row[:], evi_row[:])

        # s1 = sum(theta * ev), evs = sum(ev), via fused accumulate ops
        thev_row = singles.tile([1, NLOC], F32)
        s1_sb = singles.tile([1, 1], F32)
        nc.vector.scalar_tensor_tensor(
            thev_row[:],
            theta_row[:],
            1.0,
            evf_row[:],
            mybir.AluOpType.mult,
            mybir.AluOpType.mult,
            accum_out=s1_sb[:],
        )
        evcopy_row = singles.tile([1, NLOC], F32)
        evs_sb = singles.tile([1, 1], F32)
        nc.vector.tensor_scalar(
            evcopy_row[:],
            evf_row[:],
            0.0,
            None,
            mybir.AluOpType.add,
            mybir.AluOpType.add,
            accum_out=evs_sb[:],
        )

        # --- stage 2: log-denominator + partial loss ------------------
        # sum_i ev_i*log(denom_i + eps) == sum_i log((denom_i - 1)*ev_i + 1)
        # (+eps shift is below f32 resolution here; denom >= e^-20 + ~1)
        one_sb = singles.tile([1, 1], F32)
        nc.vector.memset(one_sb[:], 1.0)
        y_row = singles.tile([1, NLOC], F32)
        ls_sb = singles.tile([1, NB], F32)
        for ib in range(NB):
            nc.vector.scalar_tensor_tensor(
                y_row[:, ib * IB : (ib + 1) * IB],
                psums[ib][:],
                1.0,
                evf_row[:, ib * IB : (ib + 1) * IB],
                mybir.AluOpType.subtract,
                mybir.AluOpType.mult,
            )
            nc.scalar.activation(
                y_row[:, ib * IB : (ib + 1) * IB],
                y_row[:, ib * IB : (ib + 1) * IB],
                mybir.ActivationFunctionType.Ln,
                bias=one_sb[:],
                accum_out=ls_sb[:, ib : ib + 1],
            )

        sumlog_sb = singles.tile([1, 1], F32)
        nc.vector.tensor_reduce(
            sumlog_sb[:], ls_sb[:], mybir.AxisListType.X, mybir.AluOpType.add
        )
        num_sb = singles.tile([1, 1], F32)
        nc.vector.tensor_tensor(
            num_sb[:], s1_sb[:], sumlog_sb[:], mybir.AluOpType.subtract
        )

        nc.sync.dma_start(num_out[:], num_sb[:])
        nc.sync.dma_start(evs_out[:], evs_sb[:])

    _prog_cache["nc"] = nc
    return nc


def kernel(risk, time, event):
    risk = np.asarray(risk, dtype=np.float32)
    time = np.asarray(time, dtype=np.float32)
    event = np.asarray(event, dtype=np.int32)

    nc = _build_program()
    _finalize(nc)

    in_maps = []
    for c in range(NCORES):
        s = slice(c * NLOC, (c + 1) * NLOC)
        in_maps.append(
            {
                "t_full": time,
                "r_full": risk,
                "tlb": np.ascontiguousarray(
                    np.broadcast_to(time[s] * np.float32(USCALE), (P, NLOC))
                ),
                "r_loc": risk[s].copy(),
                "ev_loc": event[s].copy(),
            }
        )

    res = bass_utils.run_bass_kernel_spmd(nc, in_maps, core_ids=list(range(NCORES)))

    num = sum(float(res.results[c]["num"][0, 0]) for c in range(NCORES))
    evs = sum(float(res.results[c]["evs"][0, 0]) for c in range(NCORES))
    return np.float32(-(num / (evs + 1e-8)))


def _make_in_maps(risk, time, event):
    in_maps = []
    for c in range(NCORES):
        s = slice(c * NLOC, (c + 1) * NLOC)
        in_maps.append(
            {
                "t_full": time,
                "r_full": risk,
                "tlb": np.ascontiguousarray(
                    np.broadcast_to(time[s] * np.float32(USCALE), (P, NLOC))
                ),
                "r_loc": risk[s].copy(),
                "ev_loc": event[s].copy(),
            }
        )
    return in_maps


def profile(np_inputs, tmpdir=None):
    """Run once with NTFF tracing; returns exec_time_ns (max across cores)."""
    risk = np.asarray(np_inputs["risk"], dtype=np.float32)
    time = np.asarray(np_inputs["time"], dtype=np.float32)
    event = np.asarray(np_inputs["event"], dtype=np.int32)
    nc = _build_program()
    _finalize(nc)
    res = bass_utils.run_bass_kernel_spmd(
        nc,
        _make_in_maps(risk, time, event),
        core_ids=list(range(NCORES)),
        trace=True,
        tmpdir=tmpdir,
    )
    if res.instructions_and_trace is not None:
        print("trace:", res.instructions_and_trace[1])
    print("mean_exec_time_ns:", res.mean_exec_time_ns,
          "max core:", res.max_exec_time_core_id)
    return res.exec_time_ns
